# revision 7
# baseline (speedup 1.0000x reference)
"""Trainium2 Bass kernel for nn_BOW: emb = relu(relu(relu(bow(idx) @ W1.T + b1) @ W2.T + b2) @ W3.T + b3).

Strategy: data-parallel over batch across 8 NeuronCores (32 rows each).
The bow-histogram @ W1.T product is reformulated as an embedding-bag:
    h1[b] = b1 + sum_j W1T[idx[b, j]]
so no histogram scatter is ever materialized.  Each core indirect-DMA-gathers
its 32*512 = 16384 token rows (2 KB each) from a host-prepped fp16 table
W1T [V, 1024] (fp16 keeps the summed-row error ~1e-4 << the 2e-2 gate,
at half the HBM traffic of a bf16 hi+lo split).  Gathers are batched
GPB groups per indirect-DMA instruction (offset AP [128, GPB]) so the
~1 us fixed SWDGE cost per instruction amortizes over GPB*128 descriptors.
Each gather group of 128 tokens interleaves 4 tokens from each of the 32
rows; a static one-hot "membership" lhsT [128, 32] on the TensorEngine
reduces tokens into their rows, accumulating all 128 groups in PSUM.
fc2/fc3 are small dense matmuls done per-core on its 32 rows. No collectives.

Memory roofline: 32 MB/core of gather traffic at ~358 GB/s HBM-per-core
=> ~90 us; PE (~62 us), SWDGE gen (~21 us) hide under the DMA.
"""
import sys

try:
    import concourse.bass  # noqa: F401  (already importable in some setups)
except ImportError:
    sys.path.insert(0, "/opt/trn_rl_repo")

import numpy as np
import concourse.bass as bass
import concourse.tile as tile
import concourse.mybir as mybir
from concourse import bacc
from concourse.bass_utils import run_bass_kernel_spmd
from concourse.masks import make_identity

N_CORES = 8
B, S = 256, 512
V = 50000
M1, M2, EMB = 1024, 512, 256
RPC = B // N_CORES          # rows per core = 32
TPR = 128 // RPC            # tokens per row per gather group = 4
NG = S // TPR               # gather groups per core = 128
GPB = 1                     # gather groups batched per indirect-DMA instruction
NI = NG // GPB              # indirect-DMA instructions per rep = 16

_CACHE = {}


def _build(reps=1):
    nc = bacc.Bacc("TRN2", target_bir_lowering=False, debug=False, num_devices=N_CORES)
    f32 = mybir.dt.float32
    f16 = mybir.dt.bfloat16

    w1t = nc.dram_tensor("w1t", [V, M1], f16, kind="ExternalInput")
    w2t = nc.dram_tensor("w2t", [M1, M2], f32, kind="ExternalInput")
    w3t = nc.dram_tensor("w3t", [M2, EMB], f32, kind="ExternalInput")
    b1 = nc.dram_tensor("b1", [1, M1], f32, kind="ExternalInput")
    b2 = nc.dram_tensor("b2", [1, M2], f32, kind="ExternalInput")
    b3 = nc.dram_tensor("b3", [1, EMB], f32, kind="ExternalInput")
    gidx = nc.dram_tensor("gidx", [128, NG], mybir.dt.int32, kind="ExternalInput")
    memb = nc.dram_tensor("memb", [128, RPC], f16, kind="ExternalInput")
    emb = nc.dram_tensor("emb", [reps * RPC, EMB], f32, kind="ExternalOutput")

    with tile.TileContext(nc) as tc:
        with (
            tc.tile_pool(name="const", bufs=1) as cpool,
            tc.tile_pool(name="gath", bufs=4) as gpool,
            tc.tile_pool(name="act", bufs=1) as apool,
            tc.tile_pool(name="ph1", bufs=1, space="PSUM") as ph1_pool,
            tc.tile_pool(name="ptr", bufs=2, space="PSUM") as ptr_pool,
            tc.tile_pool(name="psm", bufs=1, space="PSUM") as psm_pool,
        ):
            # ---- constants ----
            idx_t = cpool.tile([128, NG], mybir.dt.int32)
            nc.sync.dma_start(idx_t[:], gidx[:])
            memb_t = cpool.tile([128, RPC], f16)
            nc.sync.dma_start(memb_t[:], memb[:])
            b1_t = cpool.tile([1, M1], f32)
            nc.sync.dma_start(b1_t[:], b1[:])
            b2_t = cpool.tile([1, M2], f32)
            nc.sync.dma_start(b2_t[:], b2[:])
            b3_t = cpool.tile([1, EMB], f32)
            nc.sync.dma_start(b3_t[:], b3[:])
            w2_t = cpool.tile([128, M1 // 128, M2], f32)
            for a in range(M1 // 128):
                nc.sync.dma_start(w2_t[:, a, :], w2t[a * 128:(a + 1) * 128, :])
            w3_t = cpool.tile([128, M2 // 128, EMB], f32)
            for a in range(M2 // 128):
                nc.sync.dma_start(w3_t[:, a, :], w3t[a * 128:(a + 1) * 128, :])
            ones1 = cpool.tile([1, RPC], f32)
            nc.gpsimd.memset(ones1[:], 1.0)
            ident = cpool.tile([RPC, RPC], f32)
            make_identity(nc, ident[:])

            for _rep in range(reps):
                _body(nc, tc, gpool, apool, ph1_pool, ptr_pool, psm_pool,
                      idx_t, memb_t, b1_t, b2_t, b3_t, w2_t, w3_t, ones1, ident,
                      emb[_rep * RPC:(_rep + 1) * RPC, :], w1t)

    nc.compile()
    return nc


def _body(nc, tc, gpool, apool, ph1_pool, ptr_pool, psm_pool,
          idx_t, memb_t, b1_t, b2_t, b3_t, w2_t, w3_t, ones1, ident, emb, w1t):
    f32 = mybir.dt.float32
    f16 = mybir.dt.bfloat16
    # ---- fc1: batched gather + membership matmul, accumulate in PSUM ----
    ph1a = ph1_pool.tile([RPC, 512], f32, tag="h1a")
    ph1b = ph1_pool.tile([RPC, 512], f32, tag="h1b")
    ph1 = [ph1a, ph1b]
    for k in range(NI):
        gt = gpool.tile([128, GPB * M1], f16)
        nc.gpsimd.indirect_dma_start(
            out=gt[:], out_offset=None, in_=w1t[:],
            in_offset=bass.IndirectOffsetOnAxis(
                ap=idx_t[:, k * GPB:(k + 1) * GPB], axis=0
            ),
        )
        for g in range(GPB):
            for h in range(2):
                nc.tensor.matmul(
                    ph1[h][:],
                    lhsT=memb_t[:],
                    rhs=gt[:, (2 * g + h) * 512:(2 * g + h + 1) * 512],
                    start=(k == 0 and g == 0), stop=False,
                )
    h1 = apool.tile([RPC, M1], f32)
    for h in range(2):
        nc.tensor.matmul(
            ph1[h][:], lhsT=ones1[:], rhs=b1_t[:, h * 512:(h + 1) * 512],
            start=False, stop=True,
        )
        nc.scalar.activation(
            h1[:, h * 512:(h + 1) * 512], ph1[h][:],
            mybir.ActivationFunctionType.Relu,
        )

    # ---- transpose h1 -> h1t [128, 8, RPC] ----
    h1t = apool.tile([128, M1 // 128, RPC], f32)
    for a in range(M1 // 128):
        pt = ptr_pool.tile([128, RPC], f32, tag="tr")
        nc.tensor.transpose(pt[:], h1[:, a * 128:(a + 1) * 128], ident[:])
        nc.vector.tensor_copy(h1t[:, a, :], pt[:])

    # ---- fc2 ----
    ph2 = psm_pool.tile([RPC, M2], f32, tag="h2")
    for a in range(M1 // 128):
        nc.tensor.matmul(
            ph2[:], lhsT=h1t[:, a, :], rhs=w2_t[:, a, :],
            start=(a == 0), stop=False,
        )
    nc.tensor.matmul(ph2[:], lhsT=ones1[:], rhs=b2_t[:], start=False, stop=True)
    h2 = apool.tile([RPC, M2], f32)
    nc.scalar.activation(h2[:], ph2[:], mybir.ActivationFunctionType.Relu)

    # ---- transpose h2 -> h2t [128, 4, RPC] ----
    h2t = apool.tile([128, M2 // 128, RPC], f32)
    for a in range(M2 // 128):
        pt = ptr_pool.tile([128, RPC], f32, tag="tr")
        nc.tensor.transpose(pt[:], h2[:, a * 128:(a + 1) * 128], ident[:])
        nc.vector.tensor_copy(h2t[:, a, :], pt[:])

    # ---- fc3 ----
    ph3 = psm_pool.tile([RPC, EMB], f32, tag="h3")
    for a in range(M2 // 128):
        nc.tensor.matmul(
            ph3[:], lhsT=h2t[:, a, :], rhs=w3_t[:, a, :],
            start=(a == 0), stop=False,
        )
    nc.tensor.matmul(ph3[:], lhsT=ones1[:], rhs=b3_t[:], start=False, stop=True)
    out_t = apool.tile([RPC, EMB], f32)
    nc.scalar.activation(out_t[:], ph3[:], mybir.ActivationFunctionType.Relu)
    nc.sync.dma_start(emb[:], out_t[:])


def _prep_inputs(idx, W1, b1, W2, b2, W3, b3):
    """Host-side sharding/layout prep. Returns per-core input maps."""
    idx = np.asarray(idx)
    w1t = np.ascontiguousarray(
        np.asarray(W1, dtype=np.float32).T.astype(__import__("ml_dtypes").bfloat16)
    )                                                                 # [V, M1]
    w2t = np.ascontiguousarray(np.asarray(W2, dtype=np.float32).T)   # [M1, M2]
    w3t = np.ascontiguousarray(np.asarray(W3, dtype=np.float32).T)   # [M2, EMB]
    b1r = np.asarray(b1, dtype=np.float32).reshape(1, M1)
    b2r = np.asarray(b2, dtype=np.float32).reshape(1, M2)
    b3r = np.asarray(b3, dtype=np.float32).reshape(1, EMB)
    # membership: partition p belongs to row p // TPR
    memb = (np.arange(128)[:, None] // TPR == np.arange(RPC)[None, :]).astype(
        __import__("ml_dtypes").bfloat16
    )
    in_maps = []
    for c in range(N_CORES):
        rows = idx[c * RPC:(c + 1) * RPC]                 # [RPC, S]
        # gidx[p, k] = rows[p // TPR, TPR*k + p % TPR]
        g = rows.reshape(RPC, NG, TPR).transpose(0, 2, 1)  # [RPC, TPR, NG]
        gidx = g.reshape(128, NG).astype(np.int32)
        in_maps.append({
            "w1t": w1t, "w2t": w2t, "w3t": w3t,
            "b1": b1r, "b2": b2r, "b3": b3r,
            "gidx": np.ascontiguousarray(gidx), "memb": memb,
        })
    return in_maps


def kernel(idx, W1, b1, W2, b2, W3, b3):
    if "nc" not in _CACHE:
        _CACHE["nc"] = _build()
    nc = _CACHE["nc"]
    in_maps = _prep_inputs(idx, W1, b1, W2, b2, W3, b3)
    try:
        res = run_bass_kernel_spmd(nc, in_maps, list(range(N_CORES)))
    except Exception:
        # one retry: transient device errors (wedged NeuronCore from a prior
        # crashed process) usually clear on re-execution
        res = run_bass_kernel_spmd(nc, in_maps, list(range(N_CORES)))
    return np.concatenate([res.results[c]["emb"] for c in range(N_CORES)], axis=0)


# revision 10
# speedup vs baseline: 1.1059x; 1.1059x over previous
"""Trainium2 Bass kernel for nn_BOW: emb = relu(relu(relu(bow(idx) @ W1.T + b1) @ W2.T + b2) @ W3.T + b3).

Strategy: data-parallel over batch across 8 NeuronCores (32 rows each).
fc1 is an embedding-bag: h1[b] = b1 + sum_j W1T[idx[b,j]].  Token rows are
fetched with dma_gather (one SWDGE instruction per ~2176 rows instead of one
indirect-DMA instruction per 128 rows, amortizing the ~1us fixed SWDGE cost
~17x).  dma_gather indices are int16, so the fp16 table [V, 1024] is split
into two halves of 25000 rows; the host routes each token to its half
(pure index routing - all arithmetic stays on device), pads each half's
token list to a fixed capacity CAP with dummy index 0, and emits per-group
one-hot membership matrices [128, 32] whose zero rows kill the padded
slots' contributions.  The TensorEngine reduces each gathered group of 128
rows into the 32 batch rows, accumulating in PSUM across all groups.
fc2/fc3 are small dense per-core matmuls.  No collectives.
"""
import sys

try:
    import concourse.bass  # noqa: F401  (already importable in some setups)
except ImportError:
    sys.path.insert(0, "/opt/trn_rl_repo")

import numpy as np
import concourse.bass as bass
import concourse.tile as tile
import concourse.mybir as mybir
from concourse import bacc
from concourse.bass_utils import run_bass_kernel_spmd
from concourse.masks import make_identity

N_CORES = 8
B, S = 256, 512
V = 50000
VH = V // 2                 # rows per half-table = 25000
M1, M2, EMB = 1024, 512, 256
RPC = B // N_CORES          # rows per core = 32
TPC = RPC * S               # tokens per core = 16384
CAP = 8704                  # padded token capacity per half (mean 8192, +8 sigma)
# per half-table: 8 gathers of 1024 idxs + 1 of 512 (1024 is the HW-proven
# granularity; >=2048-idx gathers desynced the device in microbenches)
CHUNKS = [1024] * 8 + [512]
NGRP = 2 * CAP // 128       # membership groups per rep = 136

_CACHE = {}


def _build(reps=1):
    nc = bacc.Bacc("TRN2", target_bir_lowering=False, debug=False, num_devices=N_CORES)
    f32 = mybir.dt.float32
    f16 = mybir.dt.float16

    wa = nc.dram_tensor("wa", [VH, M1], f16, kind="ExternalInput")
    wb = nc.dram_tensor("wb", [VH, M1], f16, kind="ExternalInput")
    w2t = nc.dram_tensor("w2t", [M1, M2], f32, kind="ExternalInput")
    w3t = nc.dram_tensor("w3t", [M2, EMB], f32, kind="ExternalInput")
    b1 = nc.dram_tensor("b1", [1, M1], f32, kind="ExternalInput")
    b2 = nc.dram_tensor("b2", [1, M2], f32, kind="ExternalInput")
    b3 = nc.dram_tensor("b3", [1, EMB], f32, kind="ExternalInput")
    gidx = nc.dram_tensor("gidx", [128, 2 * CAP // 16], mybir.dt.int16,
                          kind="ExternalInput")
    membs = nc.dram_tensor("membs", [128, NGRP * RPC], f16, kind="ExternalInput")
    emb = nc.dram_tensor("emb", [reps * RPC, EMB], f32, kind="ExternalOutput")

    with tile.TileContext(nc) as tc:
        with (
            tc.tile_pool(name="const", bufs=1) as cpool,
            tc.tile_pool(name="gath", bufs=6) as gpool,
            tc.tile_pool(name="act", bufs=1) as apool,
            tc.tile_pool(name="ph1", bufs=1, space="PSUM") as ph1_pool,
            tc.tile_pool(name="ptr", bufs=2, space="PSUM") as ptr_pool,
            tc.tile_pool(name="psm", bufs=1, space="PSUM") as psm_pool,
        ):
            # ---- constants ----
            idx_t = cpool.tile([128, 2 * CAP // 16], mybir.dt.int16)
            nc.sync.dma_start(idx_t[:], gidx[:])
            membs_t = cpool.tile([128, NGRP, RPC], f16)
            nc.sync.dma_start(membs_t[:], membs[:])
            b1_t = cpool.tile([1, M1], f32)
            nc.sync.dma_start(b1_t[:], b1[:])
            b2_t = cpool.tile([1, M2], f32)
            nc.sync.dma_start(b2_t[:], b2[:])
            b3_t = cpool.tile([1, EMB], f32)
            nc.sync.dma_start(b3_t[:], b3[:])
            w2_t = cpool.tile([128, M1 // 128, M2], f32)
            for a in range(M1 // 128):
                nc.sync.dma_start(w2_t[:, a, :], w2t[a * 128:(a + 1) * 128, :])
            w3_t = cpool.tile([128, M2 // 128, EMB], f32)
            for a in range(M2 // 128):
                nc.sync.dma_start(w3_t[:, a, :], w3t[a * 128:(a + 1) * 128, :])
            ones1 = cpool.tile([1, RPC], f32)
            nc.gpsimd.memset(ones1[:], 1.0)
            ident = cpool.tile([RPC, RPC], f32)
            make_identity(nc, ident[:])

            for _rep in range(reps):
                _body(nc, tc, gpool, apool, ph1_pool, ptr_pool, psm_pool,
                      idx_t, membs_t, b1_t, b2_t, b3_t, w2_t, w3_t, ones1, ident,
                      emb[_rep * RPC:(_rep + 1) * RPC, :], wa, wb)

    nc.compile()
    return nc


def _body(nc, tc, gpool, apool, ph1_pool, ptr_pool, psm_pool,
          idx_t, membs_t, b1_t, b2_t, b3_t, w2_t, w3_t, ones1, ident, emb, wa, wb):
    f32 = mybir.dt.float32
    f16 = mybir.dt.float16
    # ---- fc1: dma_gather + per-group membership matmuls, accumulate in PSUM ----
    ph1a = ph1_pool.tile([RPC, 512], f32, tag="h1a")
    ph1b = ph1_pool.tile([RPC, 512], f32, tag="h1b")
    ph1 = [ph1a, ph1b]
    for t, tab in ((0, wa), (1, wb)):
        off = 0
        for k, nidx in enumerate(CHUNKS):
            gt = gpool.tile([128, nidx // 128, M1], f16)
            c0 = (t * CAP + off) // 16
            nc.gpsimd.dma_gather(
                out_ap=gt[:],
                in_ap=tab[:],
                idxs_ap=idx_t[:, c0:c0 + nidx // 16],
                num_idxs=nidx,
                num_idxs_reg=nidx,
                elem_size=M1,
            )
            for c in range(nidx // 128):
                gg = (t * CAP + off) // 128 + c
                for h in range(2):
                    nc.tensor.matmul(
                        ph1[h][:],
                        lhsT=membs_t[:, gg, :],
                        rhs=gt[:, c, h * 512:(h + 1) * 512],
                        start=(t == 0 and k == 0 and c == 0), stop=False,
                    )
            off += nidx
    h1 = apool.tile([RPC, M1], f32)
    for h in range(2):
        nc.tensor.matmul(
            ph1[h][:], lhsT=ones1[:], rhs=b1_t[:, h * 512:(h + 1) * 512],
            start=False, stop=True,
        )
        nc.scalar.activation(
            h1[:, h * 512:(h + 1) * 512], ph1[h][:],
            mybir.ActivationFunctionType.Relu,
        )

    # ---- transpose h1 -> h1t [128, 8, RPC] ----
    h1t = apool.tile([128, M1 // 128, RPC], f32)
    for a in range(M1 // 128):
        pt = ptr_pool.tile([128, RPC], f32, tag="tr")
        nc.tensor.transpose(pt[:], h1[:, a * 128:(a + 1) * 128], ident[:])
        nc.vector.tensor_copy(h1t[:, a, :], pt[:])

    # ---- fc2 ----
    ph2 = psm_pool.tile([RPC, M2], f32, tag="h2")
    for a in range(M1 // 128):
        nc.tensor.matmul(
            ph2[:], lhsT=h1t[:, a, :], rhs=w2_t[:, a, :],
            start=(a == 0), stop=False,
        )
    nc.tensor.matmul(ph2[:], lhsT=ones1[:], rhs=b2_t[:], start=False, stop=True)
    h2 = apool.tile([RPC, M2], f32)
    nc.scalar.activation(h2[:], ph2[:], mybir.ActivationFunctionType.Relu)

    # ---- transpose h2 -> h2t [128, 4, RPC] ----
    h2t = apool.tile([128, M2 // 128, RPC], f32)
    for a in range(M2 // 128):
        pt = ptr_pool.tile([128, RPC], f32, tag="tr")
        nc.tensor.transpose(pt[:], h2[:, a * 128:(a + 1) * 128], ident[:])
        nc.vector.tensor_copy(h2t[:, a, :], pt[:])

    # ---- fc3 ----
    ph3 = psm_pool.tile([RPC, EMB], f32, tag="h3")
    for a in range(M2 // 128):
        nc.tensor.matmul(
            ph3[:], lhsT=h2t[:, a, :], rhs=w3_t[:, a, :],
            start=(a == 0), stop=False,
        )
    nc.tensor.matmul(ph3[:], lhsT=ones1[:], rhs=b3_t[:], start=False, stop=True)
    out_t = apool.tile([RPC, EMB], f32)
    nc.scalar.activation(out_t[:], ph3[:], mybir.ActivationFunctionType.Relu)
    nc.sync.dma_start(emb[:], out_t[:])


def _prep_inputs(idx, W1, b1, W2, b2, W3, b3):
    """Host-side sharding/layout prep. Returns per-core input maps."""
    idx = np.asarray(idx)
    w1t = np.asarray(W1, dtype=np.float32).T.astype(np.float16)       # [V, M1]
    wa = np.ascontiguousarray(w1t[:VH])
    wb = np.ascontiguousarray(w1t[VH:])
    w2t = np.ascontiguousarray(np.asarray(W2, dtype=np.float32).T)   # [M1, M2]
    w3t = np.ascontiguousarray(np.asarray(W3, dtype=np.float32).T)   # [M2, EMB]
    b1r = np.asarray(b1, dtype=np.float32).reshape(1, M1)
    b2r = np.asarray(b2, dtype=np.float32).reshape(1, M2)
    b3r = np.asarray(b3, dtype=np.float32).reshape(1, EMB)

    in_maps = []
    for c in range(N_CORES):
        rows = idx[c * RPC:(c + 1) * RPC].reshape(-1)     # [TPC] in (r, j) order
        rowid = np.repeat(np.arange(RPC), S)              # token -> batch row
        flat = np.zeros(2 * CAP, dtype=np.int16)
        mrow = np.full(2 * CAP, -1, dtype=np.int32)       # -1 = padding slot
        for t in range(2):
            sel = (rows < VH) if t == 0 else (rows >= VH)
            vals = rows[sel] - t * VH
            rids = rowid[sel]
            n = vals.shape[0]
            assert n <= CAP, f"half-table overflow: {n} > {CAP}"
            flat[t * CAP:t * CAP + n] = vals.astype(np.int16)
            mrow[t * CAP:t * CAP + n] = rids
        # wrapped int16 index layout: element m of each half at [m%16, m//16]
        wrapped = np.concatenate(
            [flat[t * CAP:(t + 1) * CAP].reshape(-1, 16).T for t in range(2)],
            axis=1,
        )                                                  # [16, 2*CAP/16]
        gidx = np.tile(wrapped, (8, 1)).astype(np.int16)   # replicate to 128
        # membership: slot m -> group m//128, partition m%128, row mrow[m]
        membs = np.zeros((128, NGRP, RPC), dtype=np.float16)
        grp = np.arange(2 * CAP) // 128
        part = np.arange(2 * CAP) % 128
        valid = mrow >= 0
        membs[part[valid], grp[valid], mrow[valid]] = 1.0
        in_maps.append({
            "wa": wa, "wb": wb, "w2t": w2t, "w3t": w3t,
            "b1": b1r, "b2": b2r, "b3": b3r,
            "gidx": np.ascontiguousarray(gidx),
            "membs": np.ascontiguousarray(membs.reshape(128, NGRP * RPC)),
        })
    return in_maps


def kernel(idx, W1, b1, W2, b2, W3, b3):
    if "nc" not in _CACHE:
        _CACHE["nc"] = _build()
    nc = _CACHE["nc"]
    in_maps = _prep_inputs(idx, W1, b1, W2, b2, W3, b3)
    try:
        res = run_bass_kernel_spmd(nc, in_maps, list(range(N_CORES)))
    except Exception:
        # one retry: transient device errors (wedged NeuronCore from a prior
        # crashed process) usually clear on re-execution
        res = run_bass_kernel_spmd(nc, in_maps, list(range(N_CORES)))
    return np.concatenate([res.results[c]["emb"] for c in range(N_CORES)], axis=0)


# revision 11
# speedup vs baseline: 1.4184x; 1.2826x over previous
"""Trainium2 Bass kernel for nn_BOW: emb = relu(relu(relu(bow(idx) @ W1.T + b1) @ W2.T + b2) @ W3.T + b3).

Strategy: data-parallel over batch across 8 NeuronCores (32 rows each).
fc1 is an embedding-bag: h1[b] = b1 + sum_j W1T[idx[b,j]].  Token rows are
fetched with dma_gather (one SWDGE instruction per ~1024 rows instead of one
indirect-DMA instruction per 128 rows, amortizing the ~2us fixed per-SWDGE-
instruction cost ~8x; measured 264us/rep -> 186us/rep on the 8-core axon
trn2).  dma_gather indices are int16, so the fp16 table [V, 1024] is split
into two halves of 25000 rows; the host routes each token to its half
(pure index routing - all arithmetic stays on device), pads each half's
token list to a fixed capacity CAP with dummy index 0, and emits per-group
one-hot membership matrices [128, 32] whose zero rows kill the padded
slots' contributions.  The TensorEngine reduces each gathered group of 128
rows into the 32 batch rows, accumulating in PSUM across all groups.
fc2/fc3 are small dense per-core matmuls.  No collectives.
"""
import sys

try:
    import concourse.bass  # noqa: F401  (already importable in some setups)
except ImportError:
    sys.path.insert(0, "/opt/trn_rl_repo")

import numpy as np
import concourse.bass as bass
import concourse.tile as tile
import concourse.mybir as mybir
from concourse import bacc
from concourse.bass_utils import run_bass_kernel_spmd
from concourse.masks import make_identity

N_CORES = 8
B, S = 256, 512
V = 50000
VH = V // 2                 # rows per half-table = 25000
M1, M2, EMB = 1024, 512, 256
RPC = B // N_CORES          # rows per core = 32
TPC = RPC * S               # tokens per core = 16384
CAP = 8704                  # padded token capacity per half (mean 8192, +8 sigma)
# per half-table: 8 gathers of 1024 idxs + 1 of 512 (1024 is the HW-proven
# granularity; >=2048-idx gathers desynced the device in microbenches)
CHUNKS = [1024] * 8 + [512]
NGRP = 2 * CAP // 128       # membership groups per rep = 136

_CACHE = {}


def _build(reps=1):
    nc = bacc.Bacc("TRN2", target_bir_lowering=False, debug=False, num_devices=N_CORES)
    f32 = mybir.dt.float32
    f16 = mybir.dt.float16

    wa = nc.dram_tensor("wa", [VH, M1], f16, kind="ExternalInput")
    wb = nc.dram_tensor("wb", [VH, M1], f16, kind="ExternalInput")
    w2t = nc.dram_tensor("w2t", [M1, M2], f32, kind="ExternalInput")
    w3t = nc.dram_tensor("w3t", [M2, EMB], f32, kind="ExternalInput")
    b1 = nc.dram_tensor("b1", [1, M1], f32, kind="ExternalInput")
    b2 = nc.dram_tensor("b2", [1, M2], f32, kind="ExternalInput")
    b3 = nc.dram_tensor("b3", [1, EMB], f32, kind="ExternalInput")
    gidx = nc.dram_tensor("gidx", [128, 2 * CAP // 16], mybir.dt.int16,
                          kind="ExternalInput")
    membs = nc.dram_tensor("membs", [128, NGRP * RPC], f16, kind="ExternalInput")
    emb = nc.dram_tensor("emb", [reps * RPC, EMB], f32, kind="ExternalOutput")

    with tile.TileContext(nc) as tc:
        with (
            tc.tile_pool(name="const", bufs=1) as cpool,
            tc.tile_pool(name="gath", bufs=6) as gpool,
            tc.tile_pool(name="act", bufs=1) as apool,
            tc.tile_pool(name="ph1", bufs=1, space="PSUM") as ph1_pool,
            tc.tile_pool(name="ptr", bufs=2, space="PSUM") as ptr_pool,
            tc.tile_pool(name="psm", bufs=1, space="PSUM") as psm_pool,
        ):
            # ---- constants ----
            idx_t = cpool.tile([128, 2 * CAP // 16], mybir.dt.int16)
            nc.sync.dma_start(idx_t[:], gidx[:])
            membs_t = cpool.tile([128, NGRP, RPC], f16)
            nc.sync.dma_start(membs_t[:], membs[:])
            b1_t = cpool.tile([1, M1], f32)
            nc.sync.dma_start(b1_t[:], b1[:])
            b2_t = cpool.tile([1, M2], f32)
            nc.sync.dma_start(b2_t[:], b2[:])
            b3_t = cpool.tile([1, EMB], f32)
            nc.sync.dma_start(b3_t[:], b3[:])
            w2_t = cpool.tile([128, M1 // 128, M2], f32)
            for a in range(M1 // 128):
                nc.sync.dma_start(w2_t[:, a, :], w2t[a * 128:(a + 1) * 128, :])
            w3_t = cpool.tile([128, M2 // 128, EMB], f32)
            for a in range(M2 // 128):
                nc.sync.dma_start(w3_t[:, a, :], w3t[a * 128:(a + 1) * 128, :])
            ones1 = cpool.tile([1, RPC], f32)
            nc.gpsimd.memset(ones1[:], 1.0)
            ident = cpool.tile([RPC, RPC], f32)
            make_identity(nc, ident[:])

            for _rep in range(reps):
                _body(nc, tc, gpool, apool, ph1_pool, ptr_pool, psm_pool,
                      idx_t, membs_t, b1_t, b2_t, b3_t, w2_t, w3_t, ones1, ident,
                      emb[_rep * RPC:(_rep + 1) * RPC, :], wa, wb)

    nc.compile()
    return nc


def _body(nc, tc, gpool, apool, ph1_pool, ptr_pool, psm_pool,
          idx_t, membs_t, b1_t, b2_t, b3_t, w2_t, w3_t, ones1, ident, emb, wa, wb):
    f32 = mybir.dt.float32
    f16 = mybir.dt.float16
    # ---- fc1: dma_gather + per-group membership matmuls, accumulate in PSUM ----
    ph1a = ph1_pool.tile([RPC, 512], f32, tag="h1a")
    ph1b = ph1_pool.tile([RPC, 512], f32, tag="h1b")
    ph1 = [ph1a, ph1b]
    for t, tab in ((0, wa), (1, wb)):
        off = 0
        for k, nidx in enumerate(CHUNKS):
            gt = gpool.tile([128, nidx // 128, M1], f16)
            c0 = (t * CAP + off) // 16
            nc.gpsimd.dma_gather(
                out_ap=gt[:],
                in_ap=tab[:],
                idxs_ap=idx_t[:, c0:c0 + nidx // 16],
                num_idxs=nidx,
                num_idxs_reg=nidx,
                elem_size=M1,
            )
            for c in range(nidx // 128):
                gg = (t * CAP + off) // 128 + c
                for h in range(2):
                    nc.tensor.matmul(
                        ph1[h][:],
                        lhsT=membs_t[:, gg, :],
                        rhs=gt[:, c, h * 512:(h + 1) * 512],
                        start=(t == 0 and k == 0 and c == 0), stop=False,
                    )
            off += nidx
    h1 = apool.tile([RPC, M1], f32)
    for h in range(2):
        nc.tensor.matmul(
            ph1[h][:], lhsT=ones1[:], rhs=b1_t[:, h * 512:(h + 1) * 512],
            start=False, stop=True,
        )
        nc.scalar.activation(
            h1[:, h * 512:(h + 1) * 512], ph1[h][:],
            mybir.ActivationFunctionType.Relu,
        )

    # ---- transpose h1 -> h1t [128, 8, RPC] ----
    h1t = apool.tile([128, M1 // 128, RPC], f32)
    for a in range(M1 // 128):
        pt = ptr_pool.tile([128, RPC], f32, tag="tr")
        nc.tensor.transpose(pt[:], h1[:, a * 128:(a + 1) * 128], ident[:])
        nc.vector.tensor_copy(h1t[:, a, :], pt[:])

    # ---- fc2 ----
    ph2 = psm_pool.tile([RPC, M2], f32, tag="h2")
    for a in range(M1 // 128):
        nc.tensor.matmul(
            ph2[:], lhsT=h1t[:, a, :], rhs=w2_t[:, a, :],
            start=(a == 0), stop=False,
        )
    nc.tensor.matmul(ph2[:], lhsT=ones1[:], rhs=b2_t[:], start=False, stop=True)
    h2 = apool.tile([RPC, M2], f32)
    nc.scalar.activation(h2[:], ph2[:], mybir.ActivationFunctionType.Relu)

    # ---- transpose h2 -> h2t [128, 4, RPC] ----
    h2t = apool.tile([128, M2 // 128, RPC], f32)
    for a in range(M2 // 128):
        pt = ptr_pool.tile([128, RPC], f32, tag="tr")
        nc.tensor.transpose(pt[:], h2[:, a * 128:(a + 1) * 128], ident[:])
        nc.vector.tensor_copy(h2t[:, a, :], pt[:])

    # ---- fc3 ----
    ph3 = psm_pool.tile([RPC, EMB], f32, tag="h3")
    for a in range(M2 // 128):
        nc.tensor.matmul(
            ph3[:], lhsT=h2t[:, a, :], rhs=w3_t[:, a, :],
            start=(a == 0), stop=False,
        )
    nc.tensor.matmul(ph3[:], lhsT=ones1[:], rhs=b3_t[:], start=False, stop=True)
    out_t = apool.tile([RPC, EMB], f32)
    nc.scalar.activation(out_t[:], ph3[:], mybir.ActivationFunctionType.Relu)
    nc.sync.dma_start(emb[:], out_t[:])


def _prep_inputs(idx, W1, b1, W2, b2, W3, b3):
    """Host-side sharding/layout prep. Returns per-core input maps."""
    idx = np.asarray(idx)
    w1t = np.asarray(W1, dtype=np.float32).T.astype(np.float16)       # [V, M1]
    wa = np.ascontiguousarray(w1t[:VH])
    wb = np.ascontiguousarray(w1t[VH:])
    w2t = np.ascontiguousarray(np.asarray(W2, dtype=np.float32).T)   # [M1, M2]
    w3t = np.ascontiguousarray(np.asarray(W3, dtype=np.float32).T)   # [M2, EMB]
    b1r = np.asarray(b1, dtype=np.float32).reshape(1, M1)
    b2r = np.asarray(b2, dtype=np.float32).reshape(1, M2)
    b3r = np.asarray(b3, dtype=np.float32).reshape(1, EMB)

    in_maps = []
    for c in range(N_CORES):
        rows = idx[c * RPC:(c + 1) * RPC].reshape(-1)     # [TPC] in (r, j) order
        rowid = np.repeat(np.arange(RPC), S)              # token -> batch row
        flat = np.zeros(2 * CAP, dtype=np.int16)
        mrow = np.full(2 * CAP, -1, dtype=np.int32)       # -1 = padding slot
        for t in range(2):
            sel = (rows < VH) if t == 0 else (rows >= VH)
            vals = rows[sel] - t * VH
            rids = rowid[sel]
            n = vals.shape[0]
            assert n <= CAP, f"half-table overflow: {n} > {CAP}"
            flat[t * CAP:t * CAP + n] = vals.astype(np.int16)
            mrow[t * CAP:t * CAP + n] = rids
        # wrapped int16 index layout: element m of each half at [m%16, m//16]
        wrapped = np.concatenate(
            [flat[t * CAP:(t + 1) * CAP].reshape(-1, 16).T for t in range(2)],
            axis=1,
        )                                                  # [16, 2*CAP/16]
        gidx = np.tile(wrapped, (8, 1)).astype(np.int16)   # replicate to 128
        # membership: slot m -> group m//128, partition m%128, row mrow[m]
        membs = np.zeros((128, NGRP, RPC), dtype=np.float16)
        grp = np.arange(2 * CAP) // 128
        part = np.arange(2 * CAP) % 128
        valid = mrow >= 0
        membs[part[valid], grp[valid], mrow[valid]] = 1.0
        in_maps.append({
            "wa": wa, "wb": wb, "w2t": w2t, "w3t": w3t,
            "b1": b1r, "b2": b2r, "b3": b3r,
            "gidx": np.ascontiguousarray(gidx),
            "membs": np.ascontiguousarray(membs.reshape(128, NGRP * RPC)),
        })
    return in_maps


def kernel(idx, W1, b1, W2, b2, W3, b3):
    if "nc" not in _CACHE:
        _CACHE["nc"] = _build()
    nc = _CACHE["nc"]
    in_maps = _prep_inputs(idx, W1, b1, W2, b2, W3, b3)
    try:
        res = run_bass_kernel_spmd(nc, in_maps, list(range(N_CORES)))
    except Exception:
        # one retry: transient device errors (wedged NeuronCore from a prior
        # crashed process) usually clear on re-execution
        res = run_bass_kernel_spmd(nc, in_maps, list(range(N_CORES)))
    return np.concatenate([res.results[c]["emb"] for c in range(N_CORES)], axis=0)
